# revision 5
# baseline (speedup 1.0000x reference)
"""Trainium2 Bass kernel for CrossModalAttention.

Reference computation (B=1, C=64, N=8192 voxels):
  two cross-attention directions (CT queries over MRI keys/values and vice
  versa), each with an 8192x8192 attention matrix, fused output projection.

Sharding: each of the 8 cores owns 1024 query voxels for BOTH directions,
computes K/V over the full sequence locally (features are only 2 MB per
modality), and produces its own (64, 1024) slice of the output through the
final projection. No collectives; the host concatenates the 8 slices.

Device algorithm ("transposed world", no on-chip transposes):
  scores^T (j,i) = matmul(lhsT=K[:, j-chunk] (64,128), rhs=Q (64,512))
  exp on ScalarE straight out of PSUM (max-subtraction skipped: |s| <= ~1.2)
  AV:   pacc (65,512) += matmul(lhsT=V^T_aug[j-chunk] (128,65), rhs=exp)
        where V^T_aug's 65th column is ones -> row 64 of pacc = softmax denom
  normalize: reciprocal of denom row, K=1 matmul broadcasts it across 64
  partitions, DVE multiply writes the (c,i)-layout fused tile directly.
"""

from contextlib import ExitStack

import numpy as np

import concourse.bass as bass
import concourse.mybir as mybir
import concourse.tile as tile
from concourse import bacc
from concourse.bass_utils import run_bass_kernel_spmd

F32 = mybir.dt.float32
C = 64          # channels
N = 8192        # voxels (8*32*32)
NCORES = 8
NQ = N // NCORES      # 1024 queries per core
IH = 512              # query block (PSUM bank width in f32)
NIH = NQ // IH        # 2
JCH = 128             # key chunk (AV contraction tile)
NJ = N // JCH         # 64
GRP = 3               # score banks per exp group (3 banks -> one wide ACT op)
VGW = 8               # vT chunks per projection group
VCOLS = NJ * (C + 1)  # vT storage: 64 chunks x 65 cols (65th col = ones)


def _emit_feat_load(nc, featp, feat_dram, name):
    subs = []
    for s in range(4):
        t = featp.tile([C + 1, 2048], F32, tag="fs", name=f"{name}{s}")
        nc.sync.dma_start(t[:], feat_dram[:, 2048 * s : 2048 * (s + 1)])
        subs.append(t)
    return subs


def _emit_q_proj(nc, tc, pools, wqk_sb, qsrc_dram, wcol, name):
    qp, sp = pools["qp"], pools["sp"]
    qsrc = qp.tile([C + 1, NQ], F32, tag="qsrc", name=f"{name}_src")
    nc.sync.dma_start(qsrc[:], qsrc_dram[:])
    q = qp.tile([C, NQ], F32, tag="q", name=name)
    for h in range(NIH):
        pq = sp.tile([C, IH], F32, tag="ps", name=f"pq_{name}{h}")
        nc.tensor.matmul(
            pq[:],
            lhsT=wqk_sb[:, wcol : wcol + C],
            rhs=qsrc[:, IH * h : IH * (h + 1)],
            start=True,
            stop=True,
        )
        nc.vector.tensor_copy(q[:, IH * h : IH * (h + 1)], pq[:])
    return q


def _emit_k_proj(nc, pools, wqk_sb, fs, wcol, name):
    kp, sp = pools["kp"], pools["sp"]
    k = kp.tile([C, N], F32, tag="k", name=name)
    for ccH in range(16):
        pk = sp.tile([C, IH], F32, tag="ps", name=f"pk_{name}{ccH}")
        nc.tensor.matmul(
            pk[:],
            lhsT=wqk_sb[:, wcol : wcol + C],
            rhs=fs[ccH // 4][:, 512 * (ccH % 4) : 512 * (ccH % 4) + 512],
            start=True,
            stop=True,
        )
        nc.vector.tensor_copy(k[:, 512 * ccH : 512 * ccH + 512], pk[:])
    return k


def _emit_v_proj(nc, pools, wv_sb, fs, wcol, name):
    vp, sp = pools["vp"], pools["sp"]
    vT = vp.tile([JCH, VCOLS], F32, tag="v", name=name)
    # whole tile = 1.0; projection copies overwrite cols 0:64 of each 65-block,
    # leaving col 64 = 1.0 (the softmax-denominator accumulator column)
    nc.gpsimd.memset(vT[:], 1.0)
    for g in range(NJ // VGW):
        pv = sp.tile([JCH, VGW * C], F32, tag="ps", name=f"pv_{name}{g}")
        for cc in range(VGW):
            j = VGW * g + cc
            s, w = j // 16, j % 16
            nc.tensor.matmul(
                pv[:, C * cc : C * (cc + 1)],
                lhsT=fs[s][:, JCH * w : JCH * (w + 1)],
                rhs=wv_sb[:, wcol : wcol + C],
                start=True,
                stop=True,
            )
        dst = vT[:, 520 * g : 520 * (g + 1)].rearrange(
            "p (c w) -> p c w", w=C + 1
        )[:, :, 0:C]
        src = pv[:].rearrange("p (c w) -> p c w", w=C)
        nc.vector.tensor_copy(dst, src)
    return vT


def _emit_attention(nc, pools, q, k, vT, fused_t, d):
    sp, pap, mp, ep, npl, ones64 = (
        pools["sp"], pools["pap"], pools["mp"], pools["ep"], pools["np"],
        pools["ones64"],
    )
    for ih in range(NIH):
        pacc = pap.tile([C + 1, IH], F32, tag="pacc", name=f"pacc{d}{ih}")
        for jg in range((NJ + GRP - 1) // GRP):
            js = list(range(GRP * jg, min(GRP * (jg + 1), NJ)))
            ps = sp.tile([JCH, GRP * IH], F32, tag="ps", name=f"ps{d}{ih}{jg}")
            for idx, j in enumerate(js):
                nc.tensor.matmul(
                    ps[:, IH * idx : IH * (idx + 1)],
                    lhsT=k[:, JCH * j : JCH * (j + 1)],
                    rhs=q[:, IH * ih : IH * (ih + 1)],
                    start=True,
                    stop=True,
                )
            et = ep.tile([JCH, GRP * IH], F32, tag="exp", name=f"et{d}{ih}{jg}")
            nc.scalar.activation(
                et[:, : IH * len(js)],
                ps[:, : IH * len(js)],
                mybir.ActivationFunctionType.Exp,
            )
            for idx, j in enumerate(js):
                voff = 520 * (j // VGW) + (C + 1) * (j % VGW)
                nc.tensor.matmul(
                    pacc[:],
                    lhsT=vT[:, voff : voff + C + 1],
                    rhs=et[:, IH * idx : IH * (idx + 1)],
                    start=(j == 0),
                    stop=(j == NJ - 1),
                )
        # normalize: fused[64d:64d+64, :] = pacc[0:64] * (1 / pacc[64])
        acc_sb = npl.tile([C + 1, IH], F32, tag="acc", name=f"acc{d}{ih}")
        nc.vector.tensor_copy(acc_sb[:], pacc[:])
        r = npl.tile([1, IH], F32, tag="r", name=f"r{d}{ih}")
        nc.vector.reciprocal(r[:], acc_sb[C : C + 1, :])
        pbc = mp.tile([C, IH], F32, tag="mp", name=f"pbc{d}{ih}")
        nc.tensor.matmul(pbc[:], lhsT=ones64[:], rhs=r[:], start=True, stop=True)
        nc.vector.tensor_mul(
            fused_t[ih][C * d : C * (d + 1), :], acc_sb[0:C, :], pbc[:]
        )


def _build_program(ctx, tc, ct, mri, qsrc_ct, qsrc_mri, wqk, wv, woT, bo, out):
    nc = tc.nc
    wpool = ctx.enter_context(tc.tile_pool(name="wpool", bufs=1))
    featp = ctx.enter_context(tc.tile_pool(name="feat", bufs=4))
    pools = {
        "qp": ctx.enter_context(tc.tile_pool(name="qp", bufs=2)),
        "kp": ctx.enter_context(tc.tile_pool(name="kp", bufs=2)),
        "vp": ctx.enter_context(tc.tile_pool(name="vp", bufs=2)),
        "ep": ctx.enter_context(tc.tile_pool(name="ep", bufs=3)),
        "np": ctx.enter_context(tc.tile_pool(name="npool", bufs=2)),
        "sp": ctx.enter_context(
            tc.tile_pool(name="spsum", bufs=2, space="PSUM")
        ),
        "pap": ctx.enter_context(
            tc.tile_pool(name="paccp", bufs=1, space="PSUM")
        ),
        "mp": ctx.enter_context(tc.tile_pool(name="mpsum", bufs=1, space="PSUM")),
    }
    fp = ctx.enter_context(tc.tile_pool(name="fusedp", bufs=2))
    op = ctx.enter_context(tc.tile_pool(name="outp", bufs=2))

    wqk_sb = wpool.tile([C + 1, 4 * C], F32, name="wqk_sb")
    nc.sync.dma_start(wqk_sb[:], wqk[:])
    wv_sb = wpool.tile([C + 1, 2 * C], F32, name="wv_sb")
    nc.sync.dma_start(wv_sb[:], wv[:])
    woT_sb = wpool.tile([2 * C, C], F32, name="woT_sb")
    nc.sync.dma_start(woT_sb[:], woT[:])
    bo_sb = wpool.tile([C, 1], F32, name="bo_sb")
    nc.sync.dma_start(bo_sb[:], bo[:])
    ones64 = wpool.tile([1, C], F32, name="ones64")
    nc.gpsimd.memset(ones64[:], 1.0)
    pools["ones64"] = ones64

    # weight packing (host side): wqk cols = [wq_ct*s | wk_mri | wq_mri*s | wk_ct]
    # wv cols = [wv_mri | wv_ct]; all with bias as the 65th row.
    fused_t = [
        fp.tile([2 * C, IH], F32, tag="fused", name=f"fused{ih}")
        for ih in range(NIH)
    ]

    # mri projections first: they feed direction 0 (CT queries over MRI K/V)
    fs_mri = _emit_feat_load(nc, featp, mri, "fmri")
    q_ct = _emit_q_proj(nc, tc, pools, wqk_sb, qsrc_ct, 0 * C, "q_ct")
    k_mri = _emit_k_proj(nc, pools, wqk_sb, fs_mri, 1 * C, "k_mri")
    vT_mri = _emit_v_proj(nc, pools, wv_sb, fs_mri, 0 * C, "vT_mri")
    q_mri = _emit_q_proj(nc, tc, pools, wqk_sb, qsrc_mri, 2 * C, "q_mri")

    # direction 0 while CT projections stream in behind it
    _emit_attention(nc, pools, q_ct, k_mri, vT_mri, fused_t, 0)

    fs_ct = _emit_feat_load(nc, featp, ct, "fct")
    k_ct = _emit_k_proj(nc, pools, wqk_sb, fs_ct, 3 * C, "k_ct")
    vT_ct = _emit_v_proj(nc, pools, wv_sb, fs_ct, 1 * C, "vT_ct")

    _emit_attention(nc, pools, q_mri, k_ct, vT_ct, fused_t, 1)

    for ih in range(NIH):
        po = pools["mp"].tile([C, IH], F32, tag="mp", name=f"po{ih}")
        nc.tensor.matmul(
            po[:], lhsT=woT_sb[:], rhs=fused_t[ih][:], start=True, stop=True
        )
        ot = op.tile([C, IH], F32, tag="ot", name=f"ot{ih}")
        nc.vector.tensor_scalar_add(ot[:], po[:], bo_sb[:])
        nc.sync.dma_start(out[:, IH * ih : IH * (ih + 1)], ot[:])


def build_bass():
    nc = bacc.Bacc("TRN2", target_bir_lowering=False, debug=False)
    ct = nc.dram_tensor("ct_feat", [C + 1, N], F32, kind="ExternalInput").ap()
    mri = nc.dram_tensor("mri_feat", [C + 1, N], F32, kind="ExternalInput").ap()
    qsrc_ct = nc.dram_tensor("qsrc_ct", [C + 1, NQ], F32, kind="ExternalInput").ap()
    qsrc_mri = nc.dram_tensor("qsrc_mri", [C + 1, NQ], F32, kind="ExternalInput").ap()
    wqk = nc.dram_tensor("wqk", [C + 1, 4 * C], F32, kind="ExternalInput").ap()
    wv = nc.dram_tensor("wv", [C + 1, 2 * C], F32, kind="ExternalInput").ap()
    woT = nc.dram_tensor("woT", [2 * C, C], F32, kind="ExternalInput").ap()
    bo = nc.dram_tensor("bo", [C, 1], F32, kind="ExternalInput").ap()
    out = nc.dram_tensor("out", [C, NQ], F32, kind="ExternalOutput").ap()

    with tile.TileContext(nc) as tc, ExitStack() as ctx:
        _build_program(ctx, tc, ct, mri, qsrc_ct, qsrc_mri, wqk, wv, woT, bo, out)
    nc.compile()
    return nc


def _aug(w, b):
    # (out,in) weight + (out,) bias -> lhsT-ready [w.T; b] of shape (in+1, out)
    return np.concatenate([w.T, b[None, :]], axis=0).astype(np.float32)


def prepare_inputs(inputs):
    scale = np.float32(1.0 / np.sqrt(C))
    ct = np.asarray(inputs["ct_features"], np.float32).reshape(C, N)
    mri = np.asarray(inputs["mri_features"], np.float32).reshape(C, N)
    ones = np.ones((1, N), np.float32)
    ct_aug = np.ascontiguousarray(np.concatenate([ct, ones], axis=0))
    mri_aug = np.ascontiguousarray(np.concatenate([mri, ones], axis=0))
    wqk = np.concatenate(
        [
            _aug(np.asarray(inputs["wq_ct"]) * scale, np.asarray(inputs["bq_ct"]) * scale),
            _aug(np.asarray(inputs["wk_mri"]), np.asarray(inputs["bk_mri"])),
            _aug(np.asarray(inputs["wq_mri"]) * scale, np.asarray(inputs["bq_mri"]) * scale),
            _aug(np.asarray(inputs["wk_ct"]), np.asarray(inputs["bk_ct"])),
        ],
        axis=1,
    )
    wv = np.concatenate(
        [
            _aug(np.asarray(inputs["wv_mri"]), np.asarray(inputs["bv_mri"])),
            _aug(np.asarray(inputs["wv_ct"]), np.asarray(inputs["bv_ct"])),
        ],
        axis=1,
    )
    woT = np.ascontiguousarray(np.asarray(inputs["wo"], np.float32).T)
    bo = np.ascontiguousarray(np.asarray(inputs["bo"], np.float32)[:, None])

    in_maps = []
    for i in range(NCORES):
        sl = slice(NQ * i, NQ * (i + 1))
        in_maps.append(
            {
                "ct_feat": ct_aug,
                "mri_feat": mri_aug,
                "qsrc_ct": np.ascontiguousarray(ct_aug[:, sl]),
                "qsrc_mri": np.ascontiguousarray(mri_aug[:, sl]),
                "wqk": wqk,
                "wv": wv,
                "woT": woT,
                "bo": bo,
            }
        )
    return in_maps


def assemble_output(results):
    out = np.concatenate([results[i]["out"] for i in range(NCORES)], axis=1)
    return out.reshape(1, C, 8, 32, 32)


_NC_CACHE = None


def _get_nc():
    global _NC_CACHE
    if _NC_CACHE is None:
        _NC_CACHE = build_bass()
    return _NC_CACHE


def kernel(**inputs):
    nc = _get_nc()
    in_maps = prepare_inputs(inputs)
    res = run_bass_kernel_spmd(nc, in_maps, list(range(NCORES)))
    return assemble_output(res.results)


if __name__ == "__main__":
    nc = build_bass()
    print("built OK")


# revision 15
# speedup vs baseline: 1.5878x; 1.5878x over previous
"""Trainium2 Bass kernel for CrossModalAttention.

Reference computation (B=1, C=64, N=8192 voxels):
  two cross-attention directions (CT queries over MRI keys/values and vice
  versa), each with an 8192x8192 attention matrix, fused output projection.

Sharding: each of the 8 cores owns 1024 query voxels for BOTH directions,
computes K/V over the full sequence locally (features are only 2 MB per
modality), and produces its own (64, 1024) slice of the output through the
final projection. No collectives; the host concatenates the 8 slices.

Device algorithm ("transposed world", no on-chip transposes):
  scores^T (j,i) = matmul(lhsT=K[:, j-chunk] (64,128), rhs=Q (64,512))
  exp on ScalarE straight out of PSUM (max-subtraction skipped: |s| <= ~1.2)
  AV:   pacc (65,512) += matmul(lhsT=V^T_aug[j-chunk] (128,65), rhs=exp)
        where V^T_aug's 65th column is ones -> row 64 of pacc = softmax denom
  normalize: reciprocal of denom row, K=1 matmul broadcasts it across 64
  partitions, DVE multiply writes the (c,i)-layout fused tile directly.
"""

from contextlib import ExitStack

import numpy as np

import concourse.bass as bass
import concourse.mybir as mybir
import concourse.tile as tile
from concourse import bacc
from concourse.bass_utils import run_bass_kernel_spmd

F32 = mybir.dt.float32
F32R = mybir.dt.float32r  # fp32 rounded to 11-bit mantissa; PE runs 4x faster
C = 64          # channels
N = 8192        # voxels (8*32*32)
NCORES = 8
NQ = N // NCORES      # 1024 queries per core
IH = 512              # query block (PSUM bank width in f32)
NIH = NQ // IH        # 2
JCH = 128             # key chunk (AV contraction tile)
NJ = N // JCH         # 64
GRP = 3               # score banks per exp group (3 banks -> one wide ACT op)
VGW = 8               # vT chunks per projection group
VCOLS = NJ * (C + 1)  # vT storage: 64 chunks x 65 cols (65th col = ones)


def _emit_feat_load(nc, featp, feat_dram, name):
    subs = []
    for s in range(4):
        t = featp.tile([C + 1, 2048], F32, tag="fs", name=f"{name}{s}")
        nc.sync.dma_start(t[:], feat_dram[:, 2048 * s : 2048 * (s + 1)])
        subs.append(t)
    return subs


def _emit_q_proj(nc, tc, pools, wqk_sb, qsrc_dram, wcol, name):
    qp, sp = pools["qp"], pools["sp"]
    qsrc = qp.tile([C + 1, NQ], F32, tag="qsrc", name=f"{name}_src")
    nc.sync.dma_start(qsrc[:], qsrc_dram[:])
    q = qp.tile([C, NQ], F32, tag="q", name=name)
    for h in range(NIH):
        pq = sp.tile([C, IH], F32, tag="ps", name=f"pq_{name}{h}")
        nc.tensor.matmul(
            pq[:],
            lhsT=wqk_sb[:, wcol : wcol + C],
            rhs=qsrc[:, IH * h : IH * (h + 1)],
            start=True,
            stop=True,
        )
        # rounded copy: q feeds the f32r scores matmul
        nc.vector.tensor_copy(q[:, IH * h : IH * (h + 1)].bitcast(F32R), pq[:])
    return q


def _emit_k_proj(nc, pools, wqk_sb, fs, wcol, name):
    kp, sp = pools["kp"], pools["sp"]
    k = kp.tile([C, N], F32, tag="k", name=name)
    for ccH in range(16):
        pk = sp.tile([C, IH], F32, tag="ps", name=f"pk_{name}{ccH}")
        nc.tensor.matmul(
            pk[:],
            lhsT=wqk_sb[:, wcol : wcol + C],
            rhs=fs[ccH // 4][:, 512 * (ccH % 4) : 512 * (ccH % 4) + 512],
            start=True,
            stop=True,
        )
        nc.vector.tensor_copy(k[:, 512 * ccH : 512 * ccH + 512].bitcast(F32R), pk[:])
    return k


def _emit_v_proj(nc, pools, wv_sb, fs, wcol, name):
    vp, sp = pools["vp"], pools["sp"]
    vT = vp.tile([JCH, VCOLS], F32, tag="v", name=name)
    # col 64 of each 65-block = 1.0 (softmax-denominator accumulator column).
    # memset can't emit f32r, so round-copy from an fp32 ones tile on DVE.
    ones_cols = vT[:].rearrange("p (j w) -> p j w", w=C + 1)[:, :, C : C + 1]
    nc.vector.tensor_copy(
        ones_cols.bitcast(F32R),
        pools["onesNJ"][:].rearrange("p (j w) -> p j w", w=1),
    )
    for g in range(NJ // VGW):
        pv = sp.tile([JCH, VGW * C], F32, tag="ps", name=f"pv_{name}{g}")
        for cc in range(VGW):
            j = VGW * g + cc
            s, w = j // 16, j % 16
            nc.tensor.matmul(
                pv[:, C * cc : C * (cc + 1)],
                lhsT=fs[s][:, JCH * w : JCH * (w + 1)],
                rhs=wv_sb[:, wcol : wcol + C],
                start=True,
                stop=True,
            )
        dst = vT[:, 520 * g : 520 * (g + 1)].rearrange(
            "p (c w) -> p c w", w=C + 1
        )[:, :, 0:C]
        src = pv[:].rearrange("p (c w) -> p c w", w=C)
        nc.vector.tensor_copy(dst.bitcast(F32R), src)
    return vT


def _emit_attention(nc, pools, q, k, vT, fused_t, d):
    sp, pap, mp, ep, npl, ones64 = (
        pools["sp"], pools["pap"], pools["mp"], pools["ep"], pools["np"],
        pools["ones64"],
    )
    for ih in range(NIH):
        pacc = pap.tile([C + 1, IH], F32, tag="pacc", name=f"pacc{d}{ih}")
        for jg in range((NJ + GRP - 1) // GRP):
            js = list(range(GRP * jg, min(GRP * (jg + 1), NJ)))
            ps = sp.tile([JCH, GRP * IH], F32, tag="ps", name=f"ps{d}{ih}{jg}")
            for idx, j in enumerate(js):
                nc.tensor.matmul(
                    ps[:, IH * idx : IH * (idx + 1)],
                    lhsT=k[:, JCH * j : JCH * (j + 1)].bitcast(F32R),
                    rhs=q[:, IH * ih : IH * (ih + 1)].bitcast(F32R),
                    start=True,
                    stop=True,
                )
            et = ep.tile([JCH, GRP * IH], F32, tag="exp", name=f"et{d}{ih}{jg}")
            nc.scalar.activation(
                et[:, : IH * len(js)].bitcast(F32R),
                ps[:, : IH * len(js)],
                mybir.ActivationFunctionType.Exp,
            )
            for idx, j in enumerate(js):
                voff = 520 * (j // VGW) + (C + 1) * (j % VGW)
                nc.tensor.matmul(
                    pacc[:],
                    lhsT=vT[:, voff : voff + C + 1].bitcast(F32R),
                    rhs=et[:, IH * idx : IH * (idx + 1)].bitcast(F32R),
                    start=(j == 0),
                    stop=(j == NJ - 1),
                )
        # normalize: fused[64d:64d+64, :] = pacc[0:64] * (1 / pacc[64])
        acc_sb = npl.tile([C + 1, IH], F32, tag="acc", name=f"acc{d}{ih}")
        nc.vector.tensor_copy(acc_sb[:], pacc[:])
        r = npl.tile([1, IH], F32, tag="r", name=f"r{d}{ih}")
        nc.vector.reciprocal(r[:], acc_sb[C : C + 1, :])
        # broadcast r across 64 partitions on GpSimd (frees the PE)
        rbc = npl.tile([C, IH], F32, tag="rbc", name=f"rbc{d}{ih}")
        nc.gpsimd.partition_broadcast(rbc[:], r[:])
        nc.vector.tensor_mul(
            fused_t[ih][C * d : C * (d + 1), :], acc_sb[0:C, :], rbc[:]
        )


def _build_program(ctx, tc, ct, mri, qsrc_ct, qsrc_mri, wqk, wv, woT, bo, out):
    nc = tc.nc
    wpool = ctx.enter_context(tc.tile_pool(name="wpool", bufs=1))
    featp = ctx.enter_context(tc.tile_pool(name="feat", bufs=4))
    pools = {
        "qp": ctx.enter_context(tc.tile_pool(name="qp", bufs=2)),
        "kp": ctx.enter_context(tc.tile_pool(name="kp", bufs=2)),
        "vp": ctx.enter_context(tc.tile_pool(name="vp", bufs=2)),
        "ep": ctx.enter_context(tc.tile_pool(name="ep", bufs=3)),
        "np": ctx.enter_context(tc.tile_pool(name="npool", bufs=2)),
        "sp": ctx.enter_context(
            tc.tile_pool(name="spsum", bufs=2, space="PSUM")
        ),
        "pap": ctx.enter_context(
            tc.tile_pool(name="paccp", bufs=1, space="PSUM")
        ),
        "mp": ctx.enter_context(tc.tile_pool(name="mpsum", bufs=1, space="PSUM")),
    }
    fp = ctx.enter_context(tc.tile_pool(name="fusedp", bufs=2))
    op = ctx.enter_context(tc.tile_pool(name="outp", bufs=2))

    wqk_sb = wpool.tile([C + 1, 4 * C], F32, name="wqk_sb")
    nc.sync.dma_start(wqk_sb[:], wqk[:])
    wv_sb = wpool.tile([C + 1, 2 * C], F32, name="wv_sb")
    nc.sync.dma_start(wv_sb[:], wv[:])
    woT_sb = wpool.tile([2 * C, C], F32, name="woT_sb")
    nc.sync.dma_start(woT_sb[:], woT[:])
    bo_sb = wpool.tile([C, 1], F32, name="bo_sb")
    nc.sync.dma_start(bo_sb[:], bo[:])
    ones64 = wpool.tile([1, C], F32, name="ones64")
    nc.gpsimd.memset(ones64[:], 1.0)
    pools["ones64"] = ones64
    onesNJ = wpool.tile([JCH, NJ], F32, name="onesNJ")
    nc.gpsimd.memset(onesNJ[:], 1.0)
    pools["onesNJ"] = onesNJ

    # weight packing (host side): wqk cols = [wq_ct*s | wk_mri | wq_mri*s | wk_ct]
    # wv cols = [wv_mri | wv_ct]; all with bias as the 65th row.
    fused_t = [
        fp.tile([2 * C, IH], F32, tag="fused", name=f"fused{ih}")
        for ih in range(NIH)
    ]

    # mri projections first: they feed direction 0 (CT queries over MRI K/V)
    fs_mri = _emit_feat_load(nc, featp, mri, "fmri")
    q_ct = _emit_q_proj(nc, tc, pools, wqk_sb, qsrc_ct, 0 * C, "q_ct")
    k_mri = _emit_k_proj(nc, pools, wqk_sb, fs_mri, 1 * C, "k_mri")
    vT_mri = _emit_v_proj(nc, pools, wv_sb, fs_mri, 0 * C, "vT_mri")
    q_mri = _emit_q_proj(nc, tc, pools, wqk_sb, qsrc_mri, 2 * C, "q_mri")

    # direction 0 while CT projections stream in behind it
    _emit_attention(nc, pools, q_ct, k_mri, vT_mri, fused_t, 0)

    fs_ct = _emit_feat_load(nc, featp, ct, "fct")
    k_ct = _emit_k_proj(nc, pools, wqk_sb, fs_ct, 3 * C, "k_ct")
    vT_ct = _emit_v_proj(nc, pools, wv_sb, fs_ct, 1 * C, "vT_ct")

    _emit_attention(nc, pools, q_mri, k_ct, vT_ct, fused_t, 1)

    for ih in range(NIH):
        po = pools["mp"].tile([C, IH], F32, tag="mp", name=f"po{ih}")
        nc.tensor.matmul(
            po[:], lhsT=woT_sb[:], rhs=fused_t[ih][:], start=True, stop=True
        )
        ot = op.tile([C, IH], F32, tag="ot", name=f"ot{ih}")
        nc.vector.tensor_scalar_add(ot[:], po[:], bo_sb[:])
        nc.sync.dma_start(out[:, IH * ih : IH * (ih + 1)], ot[:])


def build_bass():
    nc = bacc.Bacc("TRN2", target_bir_lowering=False, debug=False)
    ct = nc.dram_tensor("ct_feat", [C + 1, N], F32, kind="ExternalInput").ap()
    mri = nc.dram_tensor("mri_feat", [C + 1, N], F32, kind="ExternalInput").ap()
    qsrc_ct = nc.dram_tensor("qsrc_ct", [C + 1, NQ], F32, kind="ExternalInput").ap()
    qsrc_mri = nc.dram_tensor("qsrc_mri", [C + 1, NQ], F32, kind="ExternalInput").ap()
    wqk = nc.dram_tensor("wqk", [C + 1, 4 * C], F32, kind="ExternalInput").ap()
    wv = nc.dram_tensor("wv", [C + 1, 2 * C], F32, kind="ExternalInput").ap()
    woT = nc.dram_tensor("woT", [2 * C, C], F32, kind="ExternalInput").ap()
    bo = nc.dram_tensor("bo", [C, 1], F32, kind="ExternalInput").ap()
    out = nc.dram_tensor("out", [C, NQ], F32, kind="ExternalOutput").ap()

    with tile.TileContext(nc) as tc, ExitStack() as ctx:
        _build_program(ctx, tc, ct, mri, qsrc_ct, qsrc_mri, wqk, wv, woT, bo, out)
    nc.compile()
    return nc


def _aug(w, b):
    # (out,in) weight + (out,) bias -> lhsT-ready [w.T; b] of shape (in+1, out)
    return np.concatenate([w.T, b[None, :]], axis=0).astype(np.float32)


def prepare_inputs(inputs):
    scale = np.float32(1.0 / np.sqrt(C))
    ct = np.asarray(inputs["ct_features"], np.float32).reshape(C, N)
    mri = np.asarray(inputs["mri_features"], np.float32).reshape(C, N)
    ones = np.ones((1, N), np.float32)
    ct_aug = np.ascontiguousarray(np.concatenate([ct, ones], axis=0))
    mri_aug = np.ascontiguousarray(np.concatenate([mri, ones], axis=0))
    wqk = np.concatenate(
        [
            _aug(np.asarray(inputs["wq_ct"]) * scale, np.asarray(inputs["bq_ct"]) * scale),
            _aug(np.asarray(inputs["wk_mri"]), np.asarray(inputs["bk_mri"])),
            _aug(np.asarray(inputs["wq_mri"]) * scale, np.asarray(inputs["bq_mri"]) * scale),
            _aug(np.asarray(inputs["wk_ct"]), np.asarray(inputs["bk_ct"])),
        ],
        axis=1,
    )
    wv = np.concatenate(
        [
            _aug(np.asarray(inputs["wv_mri"]), np.asarray(inputs["bv_mri"])),
            _aug(np.asarray(inputs["wv_ct"]), np.asarray(inputs["bv_ct"])),
        ],
        axis=1,
    )
    woT = np.ascontiguousarray(np.asarray(inputs["wo"], np.float32).T)
    bo = np.ascontiguousarray(np.asarray(inputs["bo"], np.float32)[:, None])

    in_maps = []
    for i in range(NCORES):
        sl = slice(NQ * i, NQ * (i + 1))
        in_maps.append(
            {
                "ct_feat": ct_aug,
                "mri_feat": mri_aug,
                "qsrc_ct": np.ascontiguousarray(ct_aug[:, sl]),
                "qsrc_mri": np.ascontiguousarray(mri_aug[:, sl]),
                "wqk": wqk,
                "wv": wv,
                "woT": woT,
                "bo": bo,
            }
        )
    return in_maps


def assemble_output(results):
    out = np.concatenate([results[i]["out"] for i in range(NCORES)], axis=1)
    return out.reshape(1, C, 8, 32, 32)


_NC_CACHE = None


def _get_nc():
    global _NC_CACHE
    if _NC_CACHE is None:
        _NC_CACHE = build_bass()
    return _NC_CACHE


def kernel(**inputs):
    nc = _get_nc()
    in_maps = prepare_inputs(inputs)
    res = run_bass_kernel_spmd(nc, in_maps, list(range(NCORES)))
    return assemble_output(res.results)


if __name__ == "__main__":
    nc = build_bass()
    print("built OK")


# revision 17
# speedup vs baseline: 2.2623x; 1.4248x over previous
"""Trainium2 Bass kernel for CrossModalAttention.

Reference computation (B=1, C=64, N=8192 voxels):
  two cross-attention directions (CT queries over MRI keys/values and vice
  versa), each with an 8192x8192 attention matrix, fused output projection.

Sharding: each of the 8 cores owns 1024 query voxels for BOTH directions,
computes K/V over the full sequence locally (features are only 2 MB per
modality), and produces its own (64, 1024) slice of the output through the
final projection. No collectives; the host concatenates the 8 slices.

Device algorithm ("transposed world", no on-chip transposes):
  scores^T (j,i) = matmul(lhsT=K[:, j-chunk] (64,128), rhs=Q (64,512))
  exp on ScalarE straight out of PSUM (max-subtraction skipped: |s| <= ~1.2)
  AV:   pacc (65,512) += matmul(lhsT=V^T_aug[j-chunk] (128,65), rhs=exp)
        where V^T_aug's 65th column is ones -> row 64 of pacc = softmax denom
  normalize: reciprocal of denom row, GpSimd partition-broadcast, DVE multiply
  writes the (c,i)-layout fused tile directly; final projection in fp32.

Precision: matmul operands are fp16 (PE streams 1 col/cycle and fast weight
load kicks in; fp32 is 4x slower, float32r 2x, and bf16's 8-bit mantissa
loses 10x accuracy for identical speed -- all values here are far inside
fp16 range). Accumulation is always fp32 in PSUM; softmax denominator,
normalization and the final projection are fp32. Rounding errors of
q/k/exp/v average out over the 8192-key softmax: end-to-end ~1e-4.
"""

from contextlib import ExitStack

import ml_dtypes
import numpy as np

import concourse.bass as bass
import concourse.mybir as mybir
import concourse.tile as tile
from concourse import bacc
from concourse.bass_utils import run_bass_kernel_spmd

F32 = mybir.dt.float32
F16 = mybir.dt.float16
C = 64          # channels
N = 8192        # voxels (8*32*32)
NCORES = 8
NQ = N // NCORES      # 1024 queries per core
IH = 512              # query block (PSUM bank width in f32)
NIH = NQ // IH        # 2
JCH = 128             # key chunk (AV contraction tile)
NJ = N // JCH         # 64
GRP = 3               # score banks per exp group (3 banks -> one wide ACT op)
VGW = 8               # vT chunks per projection group
VCOLS = NJ * (C + 1)  # vT storage: 64 chunks x 65 cols (65th col = ones)


def _emit_feat_load(nc, featp, feat_dram, name):
    subs = []
    for s in range(4):
        t = featp.tile([C + 1, 2048], F16, tag="fs", name=f"{name}{s}")
        nc.sync.dma_start(t[:], feat_dram[:, 2048 * s : 2048 * (s + 1)])
        subs.append(t)
    return subs


def _emit_q_proj(nc, tc, pools, wqk_sb, qsrc_dram, wcol, name):
    qp, sp = pools["qp"], pools["sp"]
    qsrc = qp.tile([C + 1, NQ], F16, tag="qsrc", name=f"{name}_src")
    nc.sync.dma_start(qsrc[:], qsrc_dram[:])
    q = qp.tile([C, NQ], F16, tag="q", name=name)
    for h in range(NIH):
        pq = sp.tile([C, IH], F32, tag="ps", name=f"pq_{name}{h}")
        nc.tensor.matmul(
            pq[:],
            lhsT=wqk_sb[:, wcol : wcol + C],
            rhs=qsrc[:, IH * h : IH * (h + 1)],
            start=True,
            stop=True,
        )
        nc.vector.tensor_copy(q[:, IH * h : IH * (h + 1)], pq[:])
    return q


def _emit_k_proj(nc, pools, wqk_sb, fs, wcol, name):
    kp, sp = pools["kp"], pools["sp"]
    k = kp.tile([C, N], F16, tag="k", name=name)
    for ccH in range(16):
        pk = sp.tile([C, IH], F32, tag="ps", name=f"pk_{name}{ccH}")
        nc.tensor.matmul(
            pk[:],
            lhsT=wqk_sb[:, wcol : wcol + C],
            rhs=fs[ccH // 4][:, 512 * (ccH % 4) : 512 * (ccH % 4) + 512],
            start=True,
            stop=True,
        )
        nc.vector.tensor_copy(k[:, 512 * ccH : 512 * ccH + 512], pk[:])
    return k


def _emit_v_proj(nc, pools, wv_sb, fs, wcol, name):
    vp, sp = pools["vp"], pools["sp"]
    vT = vp.tile([JCH, VCOLS], F16, tag="v", name=name)
    # col 64 of each 65-block = 1.0: the softmax-denominator accumulator
    ones_cols = vT[:].rearrange("p (j w) -> p j w", w=C + 1)[:, :, C : C + 1]
    nc.vector.tensor_copy(
        ones_cols,
        pools["onesNJ"][:].rearrange("p (j w) -> p j w", w=1),
    )
    for g in range(NJ // VGW):
        pv = sp.tile([JCH, VGW * C], F32, tag="ps", name=f"pv_{name}{g}")
        for cc in range(VGW):
            j = VGW * g + cc
            s, w = j // 16, j % 16
            nc.tensor.matmul(
                pv[:, C * cc : C * (cc + 1)],
                lhsT=fs[s][:, JCH * w : JCH * (w + 1)],
                rhs=wv_sb[:, wcol : wcol + C],
                start=True,
                stop=True,
            )
        dst = vT[:, 520 * g : 520 * (g + 1)].rearrange(
            "p (c w) -> p c w", w=C + 1
        )[:, :, 0:C]
        src = pv[:].rearrange("p (c w) -> p c w", w=C)
        nc.vector.tensor_copy(dst, src)
    return vT


def _emit_attention(nc, pools, q, k, vT, fused_t, d):
    sp, pap, ep, npl = pools["sp"], pools["pap"], pools["ep"], pools["np"]
    for ih in range(NIH):
        pacc = pap.tile([C + 1, IH], F32, tag="pacc", name=f"pacc{d}{ih}")
        for jg in range((NJ + GRP - 1) // GRP):
            js = list(range(GRP * jg, min(GRP * (jg + 1), NJ)))
            ps = sp.tile([JCH, GRP * IH], F32, tag="ps", name=f"ps{d}{ih}{jg}")
            for idx, j in enumerate(js):
                nc.tensor.matmul(
                    ps[:, IH * idx : IH * (idx + 1)],
                    lhsT=k[:, JCH * j : JCH * (j + 1)],
                    rhs=q[:, IH * ih : IH * (ih + 1)],
                    start=True,
                    stop=True,
                )
            et = ep.tile([JCH, GRP * IH], F16, tag="exp", name=f"et{d}{ih}{jg}")
            nc.scalar.activation(
                et[:, : IH * len(js)],
                ps[:, : IH * len(js)],
                mybir.ActivationFunctionType.Exp,
            )
            for idx, j in enumerate(js):
                voff = 520 * (j // VGW) + (C + 1) * (j % VGW)
                nc.tensor.matmul(
                    pacc[:],
                    lhsT=vT[:, voff : voff + C + 1],
                    rhs=et[:, IH * idx : IH * (idx + 1)],
                    start=(j == 0),
                    stop=(j == NJ - 1),
                )
        # normalize: fused[64d:64d+64, :] = pacc[0:64] * (1 / pacc[64])
        acc_sb = npl.tile([C + 1, IH], F32, tag="acc", name=f"acc{d}{ih}")
        nc.vector.tensor_copy(acc_sb[:], pacc[:])
        r = npl.tile([1, IH], F32, tag="r", name=f"r{d}{ih}")
        nc.vector.reciprocal(r[:], acc_sb[C : C + 1, :])
        # broadcast r across 64 partitions on GpSimd (frees the PE)
        rbc = npl.tile([C, IH], F32, tag="rbc", name=f"rbc{d}{ih}")
        nc.gpsimd.partition_broadcast(rbc[:], r[:])
        nc.vector.tensor_mul(
            fused_t[ih][C * d : C * (d + 1), :], acc_sb[0:C, :], rbc[:]
        )


def _build_program(ctx, tc, ct, mri, qsrc_ct, qsrc_mri, wqk, wv, woT, bo, out):
    nc = tc.nc
    wpool = ctx.enter_context(tc.tile_pool(name="wpool", bufs=1))
    featp = ctx.enter_context(tc.tile_pool(name="feat", bufs=4))
    pools = {
        "qp": ctx.enter_context(tc.tile_pool(name="qp", bufs=2)),
        "kp": ctx.enter_context(tc.tile_pool(name="kp", bufs=2)),
        "vp": ctx.enter_context(tc.tile_pool(name="vp", bufs=2)),
        "ep": ctx.enter_context(tc.tile_pool(name="ep", bufs=3)),
        "np": ctx.enter_context(tc.tile_pool(name="npool", bufs=2)),
        "sp": ctx.enter_context(
            tc.tile_pool(name="spsum", bufs=2, space="PSUM")
        ),
        "pap": ctx.enter_context(
            tc.tile_pool(name="paccp", bufs=1, space="PSUM")
        ),
        "mp": ctx.enter_context(tc.tile_pool(name="mpsum", bufs=1, space="PSUM")),
    }
    fp = ctx.enter_context(tc.tile_pool(name="fusedp", bufs=2))
    op = ctx.enter_context(tc.tile_pool(name="outp", bufs=2))

    wqk_sb = wpool.tile([C + 1, 4 * C], F16, name="wqk_sb")
    nc.sync.dma_start(wqk_sb[:], wqk[:])
    wv_sb = wpool.tile([C + 1, 2 * C], F16, name="wv_sb")
    nc.sync.dma_start(wv_sb[:], wv[:])
    woT_sb = wpool.tile([2 * C, C], F32, name="woT_sb")
    nc.sync.dma_start(woT_sb[:], woT[:])
    bo_sb = wpool.tile([C, 1], F32, name="bo_sb")
    nc.sync.dma_start(bo_sb[:], bo[:])
    onesNJ = wpool.tile([JCH, NJ], F32, name="onesNJ")
    nc.gpsimd.memset(onesNJ[:], 1.0)
    pools["onesNJ"] = onesNJ

    # weight packing (host side): wqk cols = [wq_ct*s | wk_mri | wq_mri*s | wk_ct]
    # wv cols = [wv_mri | wv_ct]; all with bias as the 65th row.
    fused_t = [
        fp.tile([2 * C, IH], F32, tag="fused", name=f"fused{ih}")
        for ih in range(NIH)
    ]

    # mri projections first: they feed direction 0 (CT queries over MRI K/V)
    fs_mri = _emit_feat_load(nc, featp, mri, "fmri")
    q_ct = _emit_q_proj(nc, tc, pools, wqk_sb, qsrc_ct, 0 * C, "q_ct")
    k_mri = _emit_k_proj(nc, pools, wqk_sb, fs_mri, 1 * C, "k_mri")
    vT_mri = _emit_v_proj(nc, pools, wv_sb, fs_mri, 0 * C, "vT_mri")
    q_mri = _emit_q_proj(nc, tc, pools, wqk_sb, qsrc_mri, 2 * C, "q_mri")

    # direction 0 while CT projections stream in behind it
    _emit_attention(nc, pools, q_ct, k_mri, vT_mri, fused_t, 0)

    fs_ct = _emit_feat_load(nc, featp, ct, "fct")
    k_ct = _emit_k_proj(nc, pools, wqk_sb, fs_ct, 3 * C, "k_ct")
    vT_ct = _emit_v_proj(nc, pools, wv_sb, fs_ct, 1 * C, "vT_ct")

    _emit_attention(nc, pools, q_mri, k_ct, vT_ct, fused_t, 1)

    for ih in range(NIH):
        po = pools["mp"].tile([C, IH], F32, tag="mp", name=f"po{ih}")
        nc.tensor.matmul(
            po[:], lhsT=woT_sb[:], rhs=fused_t[ih][:], start=True, stop=True
        )
        ot = op.tile([C, IH], F32, tag="ot", name=f"ot{ih}")
        nc.vector.tensor_scalar_add(ot[:], po[:], bo_sb[:])
        nc.sync.dma_start(out[:, IH * ih : IH * (ih + 1)], ot[:])


def build_bass():
    nc = bacc.Bacc("TRN2", target_bir_lowering=False, debug=False)
    ct = nc.dram_tensor("ct_feat", [C + 1, N], F16, kind="ExternalInput").ap()
    mri = nc.dram_tensor("mri_feat", [C + 1, N], F16, kind="ExternalInput").ap()
    qsrc_ct = nc.dram_tensor("qsrc_ct", [C + 1, NQ], F16, kind="ExternalInput").ap()
    qsrc_mri = nc.dram_tensor("qsrc_mri", [C + 1, NQ], F16, kind="ExternalInput").ap()
    wqk = nc.dram_tensor("wqk", [C + 1, 4 * C], F16, kind="ExternalInput").ap()
    wv = nc.dram_tensor("wv", [C + 1, 2 * C], F16, kind="ExternalInput").ap()
    woT = nc.dram_tensor("woT", [2 * C, C], F32, kind="ExternalInput").ap()
    bo = nc.dram_tensor("bo", [C, 1], F32, kind="ExternalInput").ap()
    out = nc.dram_tensor("out", [C, NQ], F32, kind="ExternalOutput").ap()

    with tile.TileContext(nc) as tc, ExitStack() as ctx:
        _build_program(ctx, tc, ct, mri, qsrc_ct, qsrc_mri, wqk, wv, woT, bo, out)
    nc.compile()
    return nc


def _aug(w, b):
    # (out,in) weight + (out,) bias -> lhsT-ready [w.T; b] of shape (in+1, out)
    return np.concatenate(
        [np.asarray(w, np.float32).T, np.asarray(b, np.float32)[None, :]], axis=0
    )


def prepare_inputs(inputs):
    scale = np.float32(1.0 / np.sqrt(C))
    ct = np.asarray(inputs["ct_features"], np.float32).reshape(C, N)
    mri = np.asarray(inputs["mri_features"], np.float32).reshape(C, N)
    ones = np.ones((1, N), np.float32)
    ct_aug = np.concatenate([ct, ones], axis=0).astype(np.float16)
    mri_aug = np.concatenate([mri, ones], axis=0).astype(np.float16)
    wqk = np.concatenate(
        [
            _aug(np.asarray(inputs["wq_ct"]) * scale, np.asarray(inputs["bq_ct"]) * scale),
            _aug(inputs["wk_mri"], inputs["bk_mri"]),
            _aug(np.asarray(inputs["wq_mri"]) * scale, np.asarray(inputs["bq_mri"]) * scale),
            _aug(inputs["wk_ct"], inputs["bk_ct"]),
        ],
        axis=1,
    ).astype(np.float16)
    wv = np.concatenate(
        [_aug(inputs["wv_mri"], inputs["bv_mri"]), _aug(inputs["wv_ct"], inputs["bv_ct"])],
        axis=1,
    ).astype(np.float16)
    woT = np.ascontiguousarray(np.asarray(inputs["wo"], np.float32).T)
    bo = np.ascontiguousarray(np.asarray(inputs["bo"], np.float32)[:, None])

    in_maps = []
    for i in range(NCORES):
        sl = slice(NQ * i, NQ * (i + 1))
        in_maps.append(
            {
                "ct_feat": ct_aug,
                "mri_feat": mri_aug,
                "qsrc_ct": np.ascontiguousarray(ct_aug[:, sl]),
                "qsrc_mri": np.ascontiguousarray(mri_aug[:, sl]),
                "wqk": wqk,
                "wv": wv,
                "woT": woT,
                "bo": bo,
            }
        )
    return in_maps


def assemble_output(results):
    out = np.concatenate([results[i]["out"] for i in range(NCORES)], axis=1)
    return out.reshape(1, C, 8, 32, 32)


_NC_CACHE = None


def _get_nc():
    global _NC_CACHE
    if _NC_CACHE is None:
        _NC_CACHE = build_bass()
    return _NC_CACHE


def kernel(**inputs):
    nc = _get_nc()
    in_maps = prepare_inputs(inputs)
    res = run_bass_kernel_spmd(nc, in_maps, list(range(NCORES)))
    return assemble_output(res.results)


if __name__ == "__main__":
    nc = build_bass()
    print("built OK")


# revision 18
# speedup vs baseline: 2.5550x; 1.1294x over previous
"""Trainium2 Bass kernel for CrossModalAttention.

Reference computation (B=1, C=64, N=8192 voxels):
  two cross-attention directions (CT queries over MRI keys/values and vice
  versa), each with an 8192x8192 attention matrix, fused output projection.

Sharding: each of the 8 cores owns 1024 query voxels for BOTH directions,
computes K/V over the full sequence locally (features are only 2 MB per
modality), and produces its own (64, 1024) slice of the output through the
final projection. No collectives; the host concatenates the 8 slices.

Device algorithm ("transposed world", no transposes of large tensors):
  scores^T (j,i) = matmul(lhsT=K[:, j-chunk] (64,128), rhs=Q (64,512))
  exp on ScalarE straight out of PSUM (max-subtraction skipped: |s| <= ~1.2),
  batched 3 PSUM banks per ACTIVATE to amortize the 352-cycle overhead.
  AV flipped to out=(i,c) so the PE output partition dim is the full 128:
    att^T[i, 0:64] += matmul(lhsT=exp chunk (j,128i), rhs=V^T_aug (j,65))
  V^T_aug's 65th column is ones, so column 64 accumulates the softmax
  denominator. All four 128-query chains of one 512-query block share a
  single PSUM bank: only the first matmul uses start=True (clearing the
  bank); later chains' first writes land on has_written=0 cells and
  overwrite, which initializes them correctly.
  Normalize = per-partition reciprocal of column 64 + tensor_scalar mult
  (no cross-partition broadcasts needed), then a 128x64 PE transpose per
  subblock rebuilds the channel-major fused tile for the final projection.

Precision: matmul operands are fp16 (PE streams 1 col/cycle and fast weight
load kicks in; fp32 is 4x slower, float32r 2x, and bf16's 8-bit mantissa
loses 10x accuracy for identical speed -- all values here are far inside
fp16 range). Accumulation is always fp32 in PSUM; softmax denominator,
normalization, transposes and the final projection are fp32. Rounding
errors of q/k/exp/v average out over the 8192-key softmax: end-to-end
error ~2e-4.
"""

from contextlib import ExitStack

import numpy as np

import concourse.bass as bass
import concourse.mybir as mybir
import concourse.tile as tile
from concourse import bacc
from concourse.bass_utils import run_bass_kernel_spmd

F32 = mybir.dt.float32
F16 = mybir.dt.float16
C = 64          # channels
N = 8192        # voxels (8*32*32)
NCORES = 8
NQ = N // NCORES      # 1024 queries per core
IH = 512              # query block (PSUM bank width in f32)
NIH = NQ // IH        # 2
NSUB = IH // 128      # 4 query subblocks per block (AV lhsT width)
JCH = 128             # key chunk (AV contraction tile)
NJ = N // JCH         # 64
GRP = 3               # score banks per exp group (3 banks -> one wide ACT op)
VGW = 8               # vT chunks per projection group
VCOLS = NJ * (C + 1)  # vT storage: 64 chunks x 65 cols (65th col = ones)
NFS = 8               # feature DMA subtiles
FSW = N // NFS        # 1024 cols per subtile


def _emit_feat_load(nc, featp, feat_dram, name):
    subs = []
    for s in range(NFS):
        t = featp.tile([C + 1, FSW], F16, tag="fs", name=f"{name}{s}")
        nc.sync.dma_start(t[:], feat_dram[:, FSW * s : FSW * (s + 1)])
        subs.append(t)
    return subs


def _emit_q_proj(nc, pools, wqk_sb, qsrc_dram, wcol, name):
    qp, sp = pools["qp"], pools["sp"]
    qsrc = qp.tile([C + 1, NQ], F16, tag="qsrc", name=f"{name}_src")
    nc.sync.dma_start(qsrc[:], qsrc_dram[:])
    q = qp.tile([C, NQ], F16, tag="q", name=name)
    for h in range(NIH):
        pq = sp.tile([C, IH], F32, tag="ps", name=f"pq_{name}{h}")
        nc.tensor.matmul(
            pq[:],
            lhsT=wqk_sb[:, wcol : wcol + C],
            rhs=qsrc[:, IH * h : IH * (h + 1)],
            start=True,
            stop=True,
        )
        nc.vector.tensor_copy(q[:, IH * h : IH * (h + 1)], pq[:])
    return q


def _emit_k_proj(nc, pools, wqk_sb, fs, wcol, name):
    kp, sp = pools["kp"], pools["sp"]
    k = kp.tile([C, N], F16, tag="k", name=name)
    for ccH in range(16):
        pk = sp.tile([C, IH], F32, tag="ps", name=f"pk_{name}{ccH}")
        nc.tensor.matmul(
            pk[:],
            lhsT=wqk_sb[:, wcol : wcol + C],
            rhs=fs[ccH // 2][:, 512 * (ccH % 2) : 512 * (ccH % 2) + 512],
            start=True,
            stop=True,
        )
        nc.vector.tensor_copy(k[:, 512 * ccH : 512 * ccH + 512], pk[:])
    return k


def _emit_v_proj(nc, pools, wv_sb, fs, wcol, name):
    vp, sp = pools["vp"], pools["sp"]
    vT = vp.tile([JCH, VCOLS], F16, tag="v", name=name)
    # col 64 of each 65-block = 1.0: the softmax-denominator accumulator
    ones_cols = vT[:].rearrange("p (j w) -> p j w", w=C + 1)[:, :, C : C + 1]
    nc.vector.tensor_copy(
        ones_cols,
        pools["onesNJ"][:].rearrange("p (j w) -> p j w", w=1),
    )
    for g in range(NJ // VGW):
        pv = sp.tile([JCH, VGW * C], F32, tag="ps", name=f"pv_{name}{g}")
        for cc in range(VGW):
            j = VGW * g + cc
            s, w = j // VGW, j % VGW
            nc.tensor.matmul(
                pv[:, C * cc : C * (cc + 1)],
                lhsT=fs[s][:, JCH * w : JCH * (w + 1)],
                rhs=wv_sb[:, wcol : wcol + C],
                start=True,
                stop=True,
            )
        dst = vT[:, 520 * g : 520 * (g + 1)].rearrange(
            "p (c w) -> p c w", w=C + 1
        )[:, :, 0:C]
        src = pv[:].rearrange("p (c w) -> p c w", w=C)
        nc.vector.tensor_copy(dst, src)
    return vT


def _emit_attention(nc, pools, q, k, vT, fused_t, d):
    sp, pap, mp, ep, npl = (
        pools["sp"], pools["pap"], pools["mp"], pools["ep"], pools["np"],
    )
    identity = pools["identity"]
    W = C + 1
    for ih in range(NIH):
        # one bank holds all four (128, 65) accumulation chains
        pacc = pap.tile([JCH, NSUB * W], F32, tag="pacc", name=f"pacc{d}{ih}")
        for jg in range((NJ + GRP - 1) // GRP):
            js = list(range(GRP * jg, min(GRP * (jg + 1), NJ)))
            ps = sp.tile([JCH, GRP * IH], F32, tag="ps", name=f"ps{d}{ih}{jg}")
            for idx, j in enumerate(js):
                nc.tensor.matmul(
                    ps[:, IH * idx : IH * (idx + 1)],
                    lhsT=k[:, JCH * j : JCH * (j + 1)],
                    rhs=q[:, IH * ih : IH * (ih + 1)],
                    start=True,
                    stop=True,
                )
            et = ep.tile([JCH, GRP * IH], F16, tag="exp", name=f"et{d}{ih}{jg}")
            nc.scalar.activation(
                et[:, : IH * len(js)],
                ps[:, : IH * len(js)],
                mybir.ActivationFunctionType.Exp,
            )
            for idx, j in enumerate(js):
                voff = 520 * (j // VGW) + W * (j % VGW)
                for isub in range(NSUB):
                    nc.tensor.matmul(
                        pacc[:, W * isub : W * (isub + 1)],
                        lhsT=et[:, IH * idx + JCH * isub : IH * idx + JCH * (isub + 1)],
                        rhs=vT[:, voff : voff + W],
                        start=(j == 0 and isub == 0),
                        stop=(j == NJ - 1 and isub == NSUB - 1),
                        skip_group_check=True,
                    )
        # normalize per query (partition): r = 1 / denom-column
        r4 = npl.tile([JCH, NSUB], F32, tag="r4", name=f"r4{d}{ih}")
        nc.vector.reciprocal(
            r4[:].rearrange("p (i w) -> p i w", w=1),
            pacc[:].rearrange("p (i w) -> p i w", w=W)[:, :, C : C + 1],
        )
        attT = npl.tile([JCH, NSUB * C], F32, tag="attT", name=f"attT{d}{ih}")
        for isub in range(NSUB):
            nc.vector.tensor_scalar_mul(
                attT[:, C * isub : C * (isub + 1)],
                pacc[:, W * isub : W * isub + C],
                r4[:, isub : isub + 1],
            )
        # transpose each (128, 64) subblock back to channel-major
        pt = mp.tile([C, IH], F32, tag="mp", name=f"pt{d}{ih}")
        for isub in range(NSUB):
            nc.tensor.transpose(
                pt[:, JCH * isub : JCH * (isub + 1)],
                attT[:, C * isub : C * (isub + 1)],
                identity[:],
            )
        nc.vector.tensor_copy(fused_t[ih][C * d : C * (d + 1), :], pt[:])


def _build_program(
    ctx, tc, ct, mri, qsrc_ct, qsrc_mri, wqk, wv, woT, bo, ident, out
):
    nc = tc.nc
    wpool = ctx.enter_context(tc.tile_pool(name="wpool", bufs=1))
    featp = ctx.enter_context(tc.tile_pool(name="feat", bufs=NFS))
    pools = {
        "qp": ctx.enter_context(tc.tile_pool(name="qp", bufs=2)),
        "kp": ctx.enter_context(tc.tile_pool(name="kp", bufs=2)),
        "vp": ctx.enter_context(tc.tile_pool(name="vp", bufs=2)),
        "ep": ctx.enter_context(tc.tile_pool(name="ep", bufs=3)),
        "np": ctx.enter_context(tc.tile_pool(name="npool", bufs=2)),
        "sp": ctx.enter_context(
            tc.tile_pool(name="spsum", bufs=2, space="PSUM")
        ),
        "pap": ctx.enter_context(
            tc.tile_pool(name="paccp", bufs=1, space="PSUM")
        ),
        "mp": ctx.enter_context(tc.tile_pool(name="mpsum", bufs=1, space="PSUM")),
    }
    fp = ctx.enter_context(tc.tile_pool(name="fusedp", bufs=2))
    op = ctx.enter_context(tc.tile_pool(name="outp", bufs=2))

    wqk_sb = wpool.tile([C + 1, 4 * C], F16, name="wqk_sb")
    nc.sync.dma_start(wqk_sb[:], wqk[:])
    wv_sb = wpool.tile([C + 1, 2 * C], F16, name="wv_sb")
    nc.sync.dma_start(wv_sb[:], wv[:])
    woT_sb = wpool.tile([2 * C, C], F32, name="woT_sb")
    nc.sync.dma_start(woT_sb[:], woT[:])
    bo_sb = wpool.tile([C, 1], F32, name="bo_sb")
    nc.sync.dma_start(bo_sb[:], bo[:])
    ident_sb = wpool.tile([JCH, JCH], F32, name="ident_sb")
    nc.sync.dma_start(ident_sb[:], ident[:])
    pools["identity"] = ident_sb
    onesNJ = wpool.tile([JCH, NJ], F32, name="onesNJ")
    nc.gpsimd.memset(onesNJ[:], 1.0)
    pools["onesNJ"] = onesNJ

    # weight packing (host side): wqk cols = [wq_ct*s | wk_mri | wq_mri*s | wk_ct]
    # wv cols = [wv_mri | wv_ct]; all with bias as the 65th row.
    fused_t = [
        fp.tile([2 * C, IH], F32, tag="fused", name=f"fused{ih}")
        for ih in range(NIH)
    ]

    # mri projections first: they feed direction 0 (CT queries over MRI K/V)
    fs_mri = _emit_feat_load(nc, featp, mri, "fmri")
    q_ct = _emit_q_proj(nc, pools, wqk_sb, qsrc_ct, 0 * C, "q_ct")
    k_mri = _emit_k_proj(nc, pools, wqk_sb, fs_mri, 1 * C, "k_mri")
    vT_mri = _emit_v_proj(nc, pools, wv_sb, fs_mri, 0 * C, "vT_mri")
    q_mri = _emit_q_proj(nc, pools, wqk_sb, qsrc_mri, 2 * C, "q_mri")

    # direction 0 while CT projections stream in behind it
    _emit_attention(nc, pools, q_ct, k_mri, vT_mri, fused_t, 0)

    fs_ct = _emit_feat_load(nc, featp, ct, "fct")
    k_ct = _emit_k_proj(nc, pools, wqk_sb, fs_ct, 3 * C, "k_ct")
    vT_ct = _emit_v_proj(nc, pools, wv_sb, fs_ct, 1 * C, "vT_ct")

    _emit_attention(nc, pools, q_mri, k_ct, vT_ct, fused_t, 1)

    for ih in range(NIH):
        po = pools["mp"].tile([C, IH], F32, tag="mp", name=f"po{ih}")
        nc.tensor.matmul(
            po[:], lhsT=woT_sb[:], rhs=fused_t[ih][:], start=True, stop=True
        )
        ot = op.tile([C, IH], F32, tag="ot", name=f"ot{ih}")
        nc.vector.tensor_scalar_add(ot[:], po[:], bo_sb[:])
        nc.sync.dma_start(out[:, IH * ih : IH * (ih + 1)], ot[:])


def build_bass():
    nc = bacc.Bacc("TRN2", target_bir_lowering=False, debug=False)
    ct = nc.dram_tensor("ct_feat", [C + 1, N], F16, kind="ExternalInput").ap()
    mri = nc.dram_tensor("mri_feat", [C + 1, N], F16, kind="ExternalInput").ap()
    qsrc_ct = nc.dram_tensor("qsrc_ct", [C + 1, NQ], F16, kind="ExternalInput").ap()
    qsrc_mri = nc.dram_tensor("qsrc_mri", [C + 1, NQ], F16, kind="ExternalInput").ap()
    wqk = nc.dram_tensor("wqk", [C + 1, 4 * C], F16, kind="ExternalInput").ap()
    wv = nc.dram_tensor("wv", [C + 1, 2 * C], F16, kind="ExternalInput").ap()
    woT = nc.dram_tensor("woT", [2 * C, C], F32, kind="ExternalInput").ap()
    bo = nc.dram_tensor("bo", [C, 1], F32, kind="ExternalInput").ap()
    ident = nc.dram_tensor("ident", [JCH, JCH], F32, kind="ExternalInput").ap()
    out = nc.dram_tensor("out", [C, NQ], F32, kind="ExternalOutput").ap()

    with tile.TileContext(nc) as tc, ExitStack() as ctx:
        _build_program(
            ctx, tc, ct, mri, qsrc_ct, qsrc_mri, wqk, wv, woT, bo, ident, out
        )
    nc.compile()
    return nc


def _aug(w, b):
    # (out,in) weight + (out,) bias -> lhsT-ready [w.T; b] of shape (in+1, out)
    return np.concatenate(
        [np.asarray(w, np.float32).T, np.asarray(b, np.float32)[None, :]], axis=0
    )


def prepare_inputs(inputs):
    scale = np.float32(1.0 / np.sqrt(C))
    ct = np.asarray(inputs["ct_features"], np.float32).reshape(C, N)
    mri = np.asarray(inputs["mri_features"], np.float32).reshape(C, N)
    ones = np.ones((1, N), np.float32)
    ct_aug = np.concatenate([ct, ones], axis=0).astype(np.float16)
    mri_aug = np.concatenate([mri, ones], axis=0).astype(np.float16)
    wqk = np.concatenate(
        [
            _aug(np.asarray(inputs["wq_ct"]) * scale, np.asarray(inputs["bq_ct"]) * scale),
            _aug(inputs["wk_mri"], inputs["bk_mri"]),
            _aug(np.asarray(inputs["wq_mri"]) * scale, np.asarray(inputs["bq_mri"]) * scale),
            _aug(inputs["wk_ct"], inputs["bk_ct"]),
        ],
        axis=1,
    ).astype(np.float16)
    wv = np.concatenate(
        [_aug(inputs["wv_mri"], inputs["bv_mri"]), _aug(inputs["wv_ct"], inputs["bv_ct"])],
        axis=1,
    ).astype(np.float16)
    woT = np.ascontiguousarray(np.asarray(inputs["wo"], np.float32).T)
    bo = np.ascontiguousarray(np.asarray(inputs["bo"], np.float32)[:, None])
    ident = np.eye(JCH, dtype=np.float32)

    in_maps = []
    for i in range(NCORES):
        sl = slice(NQ * i, NQ * (i + 1))
        in_maps.append(
            {
                "ct_feat": ct_aug,
                "mri_feat": mri_aug,
                "qsrc_ct": np.ascontiguousarray(ct_aug[:, sl]),
                "qsrc_mri": np.ascontiguousarray(mri_aug[:, sl]),
                "wqk": wqk,
                "wv": wv,
                "woT": woT,
                "bo": bo,
                "ident": ident,
            }
        )
    return in_maps


def assemble_output(results):
    out = np.concatenate([results[i]["out"] for i in range(NCORES)], axis=1)
    return out.reshape(1, C, 8, 32, 32)


_NC_CACHE = None


def _get_nc():
    global _NC_CACHE
    if _NC_CACHE is None:
        _NC_CACHE = build_bass()
    return _NC_CACHE


def kernel(**inputs):
    nc = _get_nc()
    in_maps = prepare_inputs(inputs)
    res = run_bass_kernel_spmd(nc, in_maps, list(range(NCORES)))
    return assemble_output(res.results)


if __name__ == "__main__":
    nc = build_bass()
    print("built OK")


# revision 20
# speedup vs baseline: 3.0389x; 1.1894x over previous
"""Trainium2 Bass kernel for CrossModalAttention.

Reference computation (B=1, C=64, N=8192 voxels):
  two cross-attention directions (CT queries over MRI keys/values and vice
  versa), each with an 8192x8192 attention matrix, fused output projection.

Sharding: each of the 8 cores owns 1024 query voxels for BOTH directions,
computes K/V over the full sequence locally (features are only 2 MB per
modality), and produces its own (64, 1024) slice of the output through the
final projection. No collectives; the host concatenates the 8 slices.

Device algorithm ("transposed world", no transposes of large tensors):
  The K projection is folded into the query side (associativity:
  (Wk f)^T q = f^T (Wk^T q)), so scores read the fp16 features directly:
    scores^T (j,i) = matmul(lhsT=feat_aug[:, j-chunk] (65,128),
                            rhs=q''_d (65,512)),  q''_d = [Wk^T q_d; bk.q_d]
  exp on ScalarE straight out of PSUM (max-subtraction skipped: |s| <= ~1.2),
  batched 3 PSUM banks per ACTIVATE to amortize the 352-cycle overhead.
  AV is flipped to out=(i,c) so the PE output partition dim is the full 128:
    att^T[i, 0:65] += matmul(lhsT=exp chunk (j,128i), rhs=V^T_aug (j,65))
  V^T_aug = feat_aug^T @ Wv' where Wv' carries the bias row and a final
  [0..0,1] column, so column 64 of att^T accumulates the softmax
  denominator for free. All four 128-query chains of one 512-query block
  share a single PSUM bank: only the first matmul uses start=True (clears
  the bank); later chains' first writes land on has_written=0 cells and
  overwrite, which initializes them correctly.
  Normalize = per-partition reciprocal of column 64 + tensor_scalar mult,
  then a 128x64 PE transpose per subblock rebuilds the channel-major
  fused tile for the fp32 final projection.

Precision: matmul operands are fp16 (PE streams 1 col/cycle with fast
weight load; fp32 is 4x slower, float32r 2x, and bf16's 8-bit mantissa
loses 10x accuracy for identical speed -- all values here are far inside
fp16 range). Accumulation is always fp32 in PSUM; softmax denominator,
normalization, transposes and the final projection are fp32. Rounding
errors of q/k/exp/v average out over the 8192-key softmax: end-to-end
error ~2e-4.
"""

from contextlib import ExitStack

import numpy as np

import concourse.bass as bass
import concourse.mybir as mybir
import concourse.tile as tile
from concourse import bacc
from concourse.bass_utils import run_bass_kernel_spmd

F32 = mybir.dt.float32
F16 = mybir.dt.float16
C = 64          # channels
N = 8192        # voxels (8*32*32)
NCORES = 8
NQ = N // NCORES      # 1024 queries per core
IH = 512              # query block (PSUM bank width in f32)
NIH = NQ // IH        # 2
NSUB = IH // 128      # 4 query subblocks per block (AV lhsT width)
JCH = 128             # key chunk (AV contraction tile)
NJ = N // JCH         # 64
GRP = 3               # score banks per exp group (3 banks -> one wide ACT op)
VGW = 4               # vT chunks per projection group (4*65 f32 fits one bank)
W = C + 1             # 65: augmented channel dim
VCOLS = NJ * W        # vT storage: 64 chunks x 65 cols (65th col = denom ones)
NFS = 8               # feature DMA subtiles
FSW = N // NFS        # 1024 cols per subtile
JPS = FSW // JCH      # 8 j-chunks per feature subtile


def _emit_feat_load(nc, featp, feat_dram, tag, name):
    subs = []
    for s in range(NFS):
        t = featp.tile([W, FSW], F16, tag=tag, name=f"{name}{s}")
        nc.sync.dma_start(t[:], feat_dram[:, FSW * s : FSW * (s + 1)])
        subs.append(t)
    return subs


def _emit_q_proj(nc, pools, wq_sb, qsrc_dram, wcol, name):
    """q_d (64, NQ) = Wq_aug^T @ qsrc_aug (bias via the features' ones row)."""
    qp, sp = pools["qp"], pools["sp"]
    qsrc = qp.tile([W, NQ], F16, tag="qsrc", name=f"{name}_src")
    nc.sync.dma_start(qsrc[:], qsrc_dram[:])
    q = qp.tile([C, NQ], F16, tag="q", name=name)
    for h in range(NIH):
        pq = sp.tile([C, IH], F32, tag="ps", name=f"pq_{name}{h}")
        nc.tensor.matmul(
            pq[:],
            lhsT=wq_sb[:, wcol : wcol + C],
            rhs=qsrc[:, IH * h : IH * (h + 1)],
            start=True,
            stop=True,
        )
        nc.vector.tensor_copy(q[:, IH * h : IH * (h + 1)], pq[:])
    return q


def _emit_qq_proj(nc, pools, wkb_sb, q, wcol, name):
    """q''_d (65, NQ) = [Wk | bk]^T @ q_d -- the K projection folded into Q."""
    qp, sp = pools["qp"], pools["sp"]
    qq = qp.tile([W, NQ], F16, tag="qq", name=name)
    for h in range(NIH):
        pq = sp.tile([W, IH], F32, tag="ps", name=f"pqq_{name}{h}")
        nc.tensor.matmul(
            pq[:],
            lhsT=wkb_sb[:, wcol : wcol + W],
            rhs=q[:, IH * h : IH * (h + 1)],
            start=True,
            stop=True,
        )
        nc.vector.tensor_copy(qq[:, IH * h : IH * (h + 1)], pq[:])
    return qq


def _emit_v_proj(nc, pools, wv_sb, fs, wcol, name):
    """vT_aug (128j x 65) chunks = feat_aug^T @ Wv' (ones column built in)."""
    vp, sp = pools["vp"], pools["sp"]
    vT = vp.tile([JCH, VCOLS], F16, tag="v", name=name)
    for g in range(NJ // VGW):
        pv = sp.tile([JCH, VGW * W], F32, tag="ps", name=f"pv_{name}{g}")
        for cc in range(VGW):
            j = VGW * g + cc
            nc.tensor.matmul(
                pv[:, W * cc : W * (cc + 1)],
                lhsT=fs[j // JPS][:, JCH * (j % JPS) : JCH * (j % JPS + 1)],
                rhs=wv_sb[:, wcol : wcol + W],
                start=True,
                stop=True,
            )
        nc.vector.tensor_copy(vT[:, W * VGW * g : W * VGW * (g + 1)], pv[:])
    return vT


def _emit_attention(nc, pools, fa, qq, vT, fused_t, d):
    sp, pap, mp, ep, npl = (
        pools["sp"], pools["pap"], pools["mp"], pools["ep"], pools["np"],
    )
    identity = pools["identity"]
    for ih in range(NIH):
        # one bank holds all four (128, 65) accumulation chains
        pacc = pap.tile([JCH, NSUB * W], F32, tag="pacc", name=f"pacc{d}{ih}")
        for jg in range((NJ + GRP - 1) // GRP):
            js = list(range(GRP * jg, min(GRP * (jg + 1), NJ)))
            ps = sp.tile([JCH, GRP * IH], F32, tag="ps", name=f"ps{d}{ih}{jg}")
            for idx, j in enumerate(js):
                nc.tensor.matmul(
                    ps[:, IH * idx : IH * (idx + 1)],
                    lhsT=fa[j // JPS][:, JCH * (j % JPS) : JCH * (j % JPS + 1)],
                    rhs=qq[:, IH * ih : IH * (ih + 1)],
                    start=True,
                    stop=True,
                )
            et = ep.tile([JCH, GRP * IH], F16, tag="exp", name=f"et{d}{ih}{jg}")
            nc.scalar.activation(
                et[:, : IH * len(js)],
                ps[:, : IH * len(js)],
                mybir.ActivationFunctionType.Exp,
            )
            for idx, j in enumerate(js):
                for isub in range(NSUB):
                    nc.tensor.matmul(
                        pacc[:, W * isub : W * (isub + 1)],
                        lhsT=et[:, IH * idx + JCH * isub : IH * idx + JCH * (isub + 1)],
                        rhs=vT[:, W * j : W * (j + 1)],
                        start=(j == 0 and isub == 0),
                        stop=(j == NJ - 1 and isub == NSUB - 1),
                        skip_group_check=True,
                    )
        # normalize per query (partition): r = 1 / denom-column
        r4 = npl.tile([JCH, NSUB], F32, tag="r4", name=f"r4{d}{ih}")
        nc.vector.reciprocal(
            r4[:].rearrange("p (i w) -> p i w", w=1),
            pacc[:].rearrange("p (i w) -> p i w", w=W)[:, :, C : C + 1],
        )
        attT = npl.tile([JCH, NSUB * C], F32, tag="attT", name=f"attT{d}{ih}")
        for isub in range(NSUB):
            nc.vector.tensor_scalar_mul(
                attT[:, C * isub : C * (isub + 1)],
                pacc[:, W * isub : W * isub + C],
                r4[:, isub : isub + 1],
            )
        # transpose each (128, 64) subblock back to channel-major
        pt = mp.tile([C, IH], F32, tag="mp", name=f"pt{d}{ih}")
        for isub in range(NSUB):
            nc.tensor.transpose(
                pt[:, JCH * isub : JCH * (isub + 1)],
                attT[:, C * isub : C * (isub + 1)],
                identity[:],
            )
        nc.vector.tensor_copy(fused_t[ih][C * d : C * (d + 1), :], pt[:])


def _build_program(
    ctx, tc, ct, mri, qsrc_ct, qsrc_mri, wq, wkb, wv, woT, bo, ident, out
):
    nc = tc.nc
    wpool = ctx.enter_context(tc.tile_pool(name="wpool", bufs=1))
    featp = ctx.enter_context(tc.tile_pool(name="feat", bufs=NFS))
    pools = {
        "qp": ctx.enter_context(tc.tile_pool(name="qp", bufs=2)),
        "vp": ctx.enter_context(tc.tile_pool(name="vp", bufs=2)),
        "ep": ctx.enter_context(tc.tile_pool(name="ep", bufs=4)),
        "np": ctx.enter_context(tc.tile_pool(name="npool", bufs=2)),
        "sp": ctx.enter_context(
            tc.tile_pool(name="spsum", bufs=2, space="PSUM")
        ),
        "pap": ctx.enter_context(
            tc.tile_pool(name="paccp", bufs=1, space="PSUM")
        ),
        "mp": ctx.enter_context(tc.tile_pool(name="mpsum", bufs=1, space="PSUM")),
    }
    fp = ctx.enter_context(tc.tile_pool(name="fusedp", bufs=2))
    op = ctx.enter_context(tc.tile_pool(name="outp", bufs=2))

    wq_sb = wpool.tile([W, 2 * C], F16, name="wq_sb")
    nc.sync.dma_start(wq_sb[:], wq[:])
    wkb_sb = wpool.tile([C, 2 * W], F16, name="wkb_sb")
    nc.sync.dma_start(wkb_sb[:], wkb[:])
    wv_sb = wpool.tile([W, 2 * W], F16, name="wv_sb")
    nc.sync.dma_start(wv_sb[:], wv[:])
    woT_sb = wpool.tile([2 * C, C], F32, name="woT_sb")
    nc.sync.dma_start(woT_sb[:], woT[:])
    bo_sb = wpool.tile([C, 1], F32, name="bo_sb")
    nc.sync.dma_start(bo_sb[:], bo[:])
    ident_sb = wpool.tile([JCH, JCH], F32, name="ident_sb")
    nc.sync.dma_start(ident_sb[:], ident[:])
    pools["identity"] = ident_sb

    fused_t = [
        fp.tile([2 * C, IH], F32, tag="fused", name=f"fused{ih}")
        for ih in range(NIH)
    ]

    # mri side first: it feeds direction 0 (CT queries over MRI K/V)
    fs_mri = _emit_feat_load(nc, featp, mri, "fsm", "fmri")
    q_ct = _emit_q_proj(nc, pools, wq_sb, qsrc_ct, 0 * C, "q_ct")
    qq_d0 = _emit_qq_proj(nc, pools, wkb_sb, q_ct, 0 * W, "qq_d0")
    vT_mri = _emit_v_proj(nc, pools, wv_sb, fs_mri, 0 * W, "vT_mri")
    q_mri = _emit_q_proj(nc, pools, wq_sb, qsrc_mri, 1 * C, "q_mri")

    # direction 0 while CT loads/projections stream in behind it
    _emit_attention(nc, pools, fs_mri, qq_d0, vT_mri, fused_t, 0)

    fs_ct = _emit_feat_load(nc, featp, ct, "fsc", "fct")
    qq_d1 = _emit_qq_proj(nc, pools, wkb_sb, q_mri, 1 * W, "qq_d1")
    vT_ct = _emit_v_proj(nc, pools, wv_sb, fs_ct, 1 * W, "vT_ct")

    _emit_attention(nc, pools, fs_ct, qq_d1, vT_ct, fused_t, 1)

    for ih in range(NIH):
        po = pools["mp"].tile([C, IH], F32, tag="mp", name=f"po{ih}")
        nc.tensor.matmul(
            po[:], lhsT=woT_sb[:], rhs=fused_t[ih][:], start=True, stop=True
        )
        ot = op.tile([C, IH], F32, tag="ot", name=f"ot{ih}")
        nc.vector.tensor_scalar_add(ot[:], po[:], bo_sb[:])
        nc.sync.dma_start(out[:, IH * ih : IH * (ih + 1)], ot[:])


def build_bass():
    nc = bacc.Bacc("TRN2", target_bir_lowering=False, debug=False)
    ct = nc.dram_tensor("ct_feat", [W, N], F16, kind="ExternalInput").ap()
    mri = nc.dram_tensor("mri_feat", [W, N], F16, kind="ExternalInput").ap()
    qsrc_ct = nc.dram_tensor("qsrc_ct", [W, NQ], F16, kind="ExternalInput").ap()
    qsrc_mri = nc.dram_tensor("qsrc_mri", [W, NQ], F16, kind="ExternalInput").ap()
    wq = nc.dram_tensor("wq", [W, 2 * C], F16, kind="ExternalInput").ap()
    wkb = nc.dram_tensor("wkb", [C, 2 * W], F16, kind="ExternalInput").ap()
    wv = nc.dram_tensor("wv", [W, 2 * W], F16, kind="ExternalInput").ap()
    woT = nc.dram_tensor("woT", [2 * C, C], F32, kind="ExternalInput").ap()
    bo = nc.dram_tensor("bo", [C, 1], F32, kind="ExternalInput").ap()
    ident = nc.dram_tensor("ident", [JCH, JCH], F32, kind="ExternalInput").ap()
    out = nc.dram_tensor("out", [C, NQ], F32, kind="ExternalOutput").ap()

    with tile.TileContext(nc) as tc, ExitStack() as ctx:
        _build_program(
            ctx, tc, ct, mri, qsrc_ct, qsrc_mri, wq, wkb, wv, woT, bo, ident, out
        )
    nc.compile()
    return nc


def _aug(w, b):
    # (out,in) weight + (out,) bias -> lhsT-ready [w.T; b] of shape (in+1, out)
    return np.concatenate(
        [np.asarray(w, np.float32).T, np.asarray(b, np.float32)[None, :]], axis=0
    )


def _wv_pack(w, b):
    # (65, 65): [[wv.T; bv] | e_last]: extra column accumulates the denominator
    m = np.zeros((W, W), np.float32)
    m[:, :C] = _aug(w, b)
    m[C, C] = 1.0
    return m


def _wkb_pack(w, b):
    # (64, 65): [wk | bk] -- K projection folded into the query side
    return np.concatenate(
        [np.asarray(w, np.float32), np.asarray(b, np.float32)[:, None]], axis=1
    )


def prepare_inputs(inputs):
    scale = np.float32(1.0 / np.sqrt(C))
    ct = np.asarray(inputs["ct_features"], np.float32).reshape(C, N)
    mri = np.asarray(inputs["mri_features"], np.float32).reshape(C, N)
    ones = np.ones((1, N), np.float32)
    ct_aug = np.concatenate([ct, ones], axis=0).astype(np.float16)
    mri_aug = np.concatenate([mri, ones], axis=0).astype(np.float16)
    wq = np.concatenate(
        [
            _aug(np.asarray(inputs["wq_ct"]) * scale, np.asarray(inputs["bq_ct"]) * scale),
            _aug(np.asarray(inputs["wq_mri"]) * scale, np.asarray(inputs["bq_mri"]) * scale),
        ],
        axis=1,
    ).astype(np.float16)
    wkb = np.concatenate(
        [_wkb_pack(inputs["wk_mri"], inputs["bk_mri"]),
         _wkb_pack(inputs["wk_ct"], inputs["bk_ct"])],
        axis=1,
    ).astype(np.float16)
    wv = np.concatenate(
        [_wv_pack(inputs["wv_mri"], inputs["bv_mri"]),
         _wv_pack(inputs["wv_ct"], inputs["bv_ct"])],
        axis=1,
    ).astype(np.float16)
    woT = np.ascontiguousarray(np.asarray(inputs["wo"], np.float32).T)
    bo = np.ascontiguousarray(np.asarray(inputs["bo"], np.float32)[:, None])
    ident = np.eye(JCH, dtype=np.float32)

    in_maps = []
    for i in range(NCORES):
        sl = slice(NQ * i, NQ * (i + 1))
        in_maps.append(
            {
                "ct_feat": ct_aug,
                "mri_feat": mri_aug,
                "qsrc_ct": np.ascontiguousarray(ct_aug[:, sl]),
                "qsrc_mri": np.ascontiguousarray(mri_aug[:, sl]),
                "wq": wq,
                "wkb": wkb,
                "wv": wv,
                "woT": woT,
                "bo": bo,
                "ident": ident,
            }
        )
    return in_maps


def assemble_output(results):
    out = np.concatenate([results[i]["out"] for i in range(NCORES)], axis=1)
    return out.reshape(1, C, 8, 32, 32)


_NC_CACHE = None


def _get_nc():
    global _NC_CACHE
    if _NC_CACHE is None:
        _NC_CACHE = build_bass()
    return _NC_CACHE


def kernel(**inputs):
    nc = _get_nc()
    in_maps = prepare_inputs(inputs)
    res = run_bass_kernel_spmd(nc, in_maps, list(range(NCORES)))
    return assemble_output(res.results)


if __name__ == "__main__":
    nc = build_bass()
    print("built OK")
